# revision 32
# baseline (speedup 1.0000x reference)
"""Multi-query attention (nn_Attention) Trainium2 Bass kernel, 8-core SPMD.

Reference computation (fp32):
    q = einsum('bnd,hde->bhne', x, Wq) * dh**-0.5
    k, v = split(x @ Wkv)                      # shared across heads (MQA)
    out = softmax(q @ k^T) @ v                 # per head
    out = concat_heads(out) @ Wout

Shapes: x [2,2048,1024], Wq [16,1024,64], Wkv [1024,128], Wout [1024,1024].

Sharding: core = b*4 + g handles batch b and heads [4g, 4g+4). Wout is split
along its input (inner) dim, so each core produces a partial [2048,1024]
output; the host sums the 4 partials per batch.

Design notes: all operands bf16 (fp32 PSUM accumulate); x transposed +
bf16-cast on the host (same host-packing category as the weight packing), so
there are no on-device transposes at all. v is projected directly in natural
[j, e] layout ([128 j, 64 e] tiles). Attention j-loop: per 128-key tile, two
K=64 quadrant sims into a 2-bank PSUM tile, one ACT exp (scale folded; this
is the pacing engine — exp N is PSUM-bank-capped at 1024 elems/partition, so
its 128 x ~1.08us instruction train is the kernel's floor), and two skewed
accumulating matmuls with lhsT=v_aug into a [65, 2, 512] PSUM accumulator
(row 64 = softmax denominators via the ones column). All (i-tile, head-pair)
j-loops form one continuous software pipeline across chunk boundaries; qT
projections for the next chunk are dripped one K-chunk per jt (so no >0.3us
PE lump ever blocks the exp train), and normalize tails + output projections
are deferred and drip-fed into the pipeline. The last i-tile's pair-0
output-projection chunk is pre-computed into SBUF partials during the final
j-loop so the tail is only chunk-1 matmuls + adds. PSUM budget (8 banks):
sim 2x2 + accumulator 2 + small 2. Input DMAs: sync ring carries the
critical path (each DMA has ~2us completion latency and serializes per
ring), scalar ring prefetches the rest.
"""

import os

import numpy as np

import concourse.mybir as mybir
import concourse.tile as tile
from concourse import bacc
from concourse.bass_utils import run_bass_kernel_spmd
from concourse.dve_ops import RECIP_APPROX_FAST_CONSTS, RECIPROCAL_APPROX_FAST

DIM = 1024
DIM_HEAD = 64
HEADS = 16
SCALE = DIM_HEAD**-0.5
B = 2
N = 2048
N_CORES = 8
HEADS_PER_CORE = HEADS // 4  # 4 head-groups across cores

P = 128
KT = DIM // P  # 8 contraction tiles
NT = N // P  # 16 key tiles of 128
IT = N // 512  # 4 i-tiles of 512
PAIRS = HEADS_PER_CORE // 2  # 2 head pairs
INNER = HEADS_PER_CORE * DIM_HEAD  # 256 per-core inner dim
CHUNKS = INNER // P  # 2 chunks of the inner dim


def _build():
    f32 = mybir.dt.float32
    f32r = mybir.dt.float32r
    bf16 = mybir.dt.bfloat16
    Exp = mybir.ActivationFunctionType.Exp

    nc = bacc.Bacc("TRN2", target_bir_lowering=False, debug=False,
                   enable_asserts=False)

    # Host-packed layouts (see _prep_in_maps): partition-major, contiguous.
    xt_d = nc.dram_tensor("xt", [P, IT, KT, 512], bf16, kind="ExternalInput")
    boot_d = nc.dram_tensor("boot", [P, KT * P + KT * 512], bf16,
                            kind="ExternalInput")
    wv_d = nc.dram_tensor("wv", [P, KT, 64], bf16, kind="ExternalInput")
    wq_d = nc.dram_tensor("wq", [P, KT, INNER], bf16, kind="ExternalInput")
    wout_d = nc.dram_tensor("wout", [P, CHUNKS, DIM], bf16, kind="ExternalInput")
    out_d = nc.dram_tensor("out", [N, DIM], bf16, kind="ExternalOutput")

    with tile.TileContext(nc) as tc:
        with (
            tc.tile_pool(name="const", bufs=1) as const,
            tc.tile_pool(name="w", bufs=1) as w,
            tc.tile_pool(name="big", bufs=1) as big,
            tc.tile_pool(name="expp", bufs=6) as expp,
            tc.tile_pool(name="small", bufs=4) as small,
            tc.tile_pool(name="rbcp", bufs=3) as rbcp,
            tc.tile_pool(name="outp", bufs=5) as outp,
            tc.tile_pool(name="ps_small", bufs=2, space="PSUM") as ps_small,
            tc.tile_pool(name="ps_sim", bufs=2, space="PSUM") as ps_sim,
            tc.tile_pool(name="ps_acc", bufs=1, space="PSUM") as ps_acc,
        ):
            xt_sb = w.tile([P, IT, KT, 512], bf16)
            boot_sb = w.tile([P, KT * P + KT * 512], bf16)
            wv_sb = w.tile([P, KT, 64], bf16)
            wq_sb = w.tile([P, KT, INNER], bf16)
            wout_sb = w.tile([P, CHUNKS, DIM], bf16)

            # Input DMAs split across both HWDGE rings. Sync ring carries the
            # critical path (kv weights + first x half-block); scalar ring
            # (idle during the prologue) streams the rest concurrently.
            nc.sync.dma_start(boot_sb[:], boot_d[:])
            nc.scalar.dma_start(wq_sb[:], wq_d[:])
            nc.scalar.dma_start(wv_sb[:], wv_d[:])
            nc.scalar.dma_start(xt_sb[:, 1], xt_d[:, 1])
            nc.scalar.dma_start(xt_sb[:, 2], xt_d[:, 2])
            nc.scalar.dma_start(xt_sb[:, 3], xt_d[:, 3])
            nc.scalar.dma_start(wout_sb[:], wout_d[:])

            onescol = const.tile([P, 1], f32)
            nc.gpsimd.memset(onescol[:], 1.0)
            ones65f = const.tile([65, 64], f32)
            nc.gpsimd.memset(ones65f[:], 1.0)
            ones65 = const.tile([65, 64], bf16)
            nc.vector.tensor_copy(ones65[64:65, :], ones65f[64:65, :])

            kT2 = big.tile([P, N], bf16)
            v_aug = big.tile([P, NT, 65], bf16)
            nc.vector.tensor_copy(
                v_aug[:, :, 64:65], onescol[:, None, :].to_broadcast((P, NT, 1))
            )
            qT = big.tile([P, PAIRS, N], bf16)
            oTn = big.tile([P, CHUNKS, N], bf16)
            rc = RECIP_APPROX_FAST_CONSTS

            def tagof(pool):
                return "sim" if pool is ps_sim else "pss"

            def wkk_ap(kt):
                return boot_sb[:, kt * P:(kt + 1) * P]

            def xt_ap(g, kt, c0=0, c1=512):
                if g == 0:
                    base = KT * P + kt * 512
                    return boot_sb[:, base + c0:base + c1]
                return xt_sb[:, g, kt, c0:c1]

            def kv_unit(g, pool):
                """kT for key block g, duplicated to both partition halves
                via the host-packed [Wk|Wk] weight."""
                isl = slice(g * 512, (g + 1) * 512)
                psk = pool.tile([P, 512], f32, tag=tagof(pool), name=f"psk{g}")
                for kt in range(KT):
                    nc.tensor.matmul(
                        psk[:],
                        wkk_ap(kt),
                        xt_ap(g, kt),
                        start=(kt == 0),
                        stop=(kt == KT - 1),
                    )
                nc.vector.tensor_copy(kT2[:, isl], psk[:])

            def v_unit(jt, pool):
                """v for key tile jt in natural [j, e] layout (no transpose)."""
                g, r = divmod(jt, 4)
                psvf = pool.tile([P, 512], f32, tag=tagof(pool), name=f"psv{jt}")
                psv = psvf[:, 0:64]
                for kt in range(KT):
                    nc.tensor.matmul(
                        psv.opt(),
                        xt_ap(g, kt, r * P, (r + 1) * P),
                        wv_sb[:, kt, :],
                        start=(kt == 0),
                        stop=(kt == KT - 1),
                    )
                nc.vector.tensor_copy(v_aug[:, jt, 0:64], psv.opt())

            def qt_proj(p, it, pool):
                isl = slice(it * 512, (it + 1) * 512)
                psq = pool.tile([P, 512], f32, tag=tagof(pool), name=f"psq{p}_{it}")
                for kt in range(KT):
                    nc.tensor.matmul(
                        psq[:],
                        wq_sb[:, kt, p * P:(p + 1) * P],
                        xt_ap(it, kt),
                        start=(kt == 0),
                        stop=(kt == KT - 1),
                    )
                nc.vector.tensor_copy(qT[:, p, isl], psq[:])

            pending = []  # deferred normalize tails / outproj groups
            qtq = []  # in-flight split qt projection chunks

            def qt_proj_split(p, it):
                """Queue a qT projection as four 2-kt chunks dripped one per
                jt. While chunks are in flight they hold the ps_small slot, so
                pending pops are suppressed (emit_jt) to keep the in-order PE
                stream deadlock-free."""
                state = {}

                def chunk(kt, p=p, it=it):
                    isl = slice(it * 512, (it + 1) * 512)
                    if kt == 0:
                        state["psq"] = ps_small.tile(
                            [P, 512], f32, tag="pss", name=f"psqs{p}_{it}"
                        )
                    psq = state["psq"]
                    nc.tensor.matmul(
                        psq[:],
                        wq_sb[:, kt, p * P:(p + 1) * P],
                        xt_ap(it, kt),
                        start=(kt == 0),
                        stop=(kt == KT - 1),
                    )
                    if kt == KT - 1:
                        nc.vector.tensor_copy(qT[:, p, isl], psq[:])

                for kt in range(KT):
                    qtq.append(lambda kt=kt: chunk(kt))

            skewq = []

            def flush_skew():
                while skewq:
                    skewq.pop(0)()

            def emit_jt(it, p, po, jt):
                isl = slice(it * 512, (it + 1) * 512)
                jsl = slice(jt * P, (jt + 1) * P)
                pss = ps_sim.tile([P, 2, 512], f32, tag="sim")
                for h in range(2):
                    nc.tensor.matmul(
                        pss[:, h, :],
                        kT2[64 * h:64 * (h + 1), jsl],
                        qT[64 * h:64 * (h + 1), p, isl],
                        tile_position=(64 * h, 0),
                    )
                et = expp.tile([P, 2, 512], bf16, tag="exp")
                nc.scalar.activation(et[:], pss[:], Exp, scale=SCALE)
                # Defer the flush at the loop's last jt: the next chunk's
                # sims then issue ahead of the two trailing oT pairs, so the
                # exp train never waits at a chunk boundary.
                if jt != NT - 1:
                    flush_skew()

                def do_oT(po=po, jt=jt, et=et):
                    for h in range(2):
                        nc.tensor.matmul(
                            po[h][:],
                            v_aug[:, jt, :],
                            et[:, h, :],
                            start=(jt == 0),
                            stop=(jt == NT - 1),
                        )

                skewq.append(do_oT)
                if qtq:
                    qtq.pop(0)()
                elif pending and (jt % 2 == 1 or len(pending) > 6):
                    pending.pop(0)()

            def alloc_po(it, p):
                pot = ps_acc.tile([65, 2, 512], f32, tag="po",
                                  name=f"po_{p}_{it}")
                return [pot[:, 0, :], pot[:, 1, :]]

            def norm_tail(it, p, h, pos):
                isl = slice(it * 512, (it + 1) * 512)
                psbf = ps_small.tile([P, 512], f32, tag="pss",
                                     name=f"psb_{p}_{it}_{h}")
                psb = psbf[0:64, :]
                nc.tensor.matmul(psb.opt(), ones65[64:65, :], pos[64:65, :])
                rbc = rbcp.tile([64, 512], f32r, tag="rbc")
                nc.vector._custom_dve(
                    RECIPROCAL_APPROX_FAST,
                    out=rbc[:],
                    in0=psb.opt(),
                    s0=rc["s0"],
                    s1=rc["s1"],
                    imm2=rc["imm2"],
                )
                nc.vector.tensor_tensor(
                    oTn[64 * h:64 * (h + 1), p, isl],
                    pos[0:64, :],
                    rbc[:],
                    mybir.AluOpType.mult,
                )

            def emit_normalize(it, p, po, defer=True):
                """Copy the accumulators to SBUF (freeing the PSUM banks,
                h=0 first so the next chunk's oT can start), then normalize
                each head; tails deferred into the pipeline. On the final
                (non-deferred) call ACT is idle, so it takes the h=0 copy."""
                poss = []
                for h in range(2):
                    pos = small.tile([65, 512], bf16, tag="pos",
                                     name=f"pos{h}_{p}_{it}")
                    if not defer and h == 0:
                        nc.scalar.copy(pos[:], po[h][:])
                    else:
                        nc.vector.tensor_copy(pos[:], po[h][:])
                    if defer:
                        pending.append(
                            lambda pool=None, it=it, p=p, h=h, pos=pos:
                                norm_tail(it, p, h, pos)
                        )
                    else:
                        norm_tail(it, p, h, pos)
                    poss.append(pos)
                return poss

            os_tiles = {}
            part_tiles = {}

            def outproj_c0(itt, dh, pool=None):
                """Pair-0 chunk of a last-it output projection, evacuated to
                SBUF so the chunk-1 matmul + add is all that remains at the
                tail."""
                dsl = slice(dh * 512, (dh + 1) * 512)
                if pool is None:
                    pool = ps_small
                pso = pool.tile([P, 512], f32, tag=tagof(pool),
                                name=f"psc_{itt}_{dh}")
                nc.tensor.matmul(pso[:], oTn[:, 0, itt * P:(itt + 1) * P],
                                 wout_sb[:, 0, dsl], start=True, stop=True)
                pt = small.tile([P, 512], f32r, tag="part", bufs=8,
                                name=f"pt_{itt}_{dh}")
                nc.vector.tensor_copy(pt[:], pso[:])
                part_tiles[(itt, dh)] = pt

            def outproj_fin(itt, dh, pool=None):
                dsl = slice(dh * 512, (dh + 1) * 512)
                if pool is None:
                    pool = ps_small
                pso = pool.tile([P, 512], f32, tag=tagof(pool),
                                name=f"psf_{itt}_{dh}")
                nc.tensor.matmul(pso[:], oTn[:, 1, itt * P:(itt + 1) * P],
                                 wout_sb[:, 1, dsl], start=True, stop=True)
                if dh == 0:
                    os_tiles[itt] = outp.tile([P, DIM], bf16, tag="os",
                                              name=f"os_{itt}")
                os_ = os_tiles[itt]
                nc.vector.tensor_tensor(os_[:, dsl], pso[:],
                                        part_tiles[(itt, dh)][:],
                                        mybir.AluOpType.add)
                if dh == 1:
                    nc.sync.dma_start(out_d[itt * P:(itt + 1) * P, :], os_[:])

            def outproj_group(itt, dh, pool=None):
                dsl = slice(dh * 512, (dh + 1) * 512)
                if pool is None:
                    pool = ps_small
                pso = pool.tile([P, 512], f32, tag=tagof(pool),
                                name=f"pso_{itt}_{dh}")
                for c in range(CHUNKS):
                    nc.tensor.matmul(
                        pso[:],
                        oTn[:, c, itt * P:(itt + 1) * P],
                        wout_sb[:, c, dsl],
                        start=(c == 0),
                        stop=(c == CHUNKS - 1),
                    )
                if dh == 0:
                    os_tiles[itt] = outp.tile([P, DIM], bf16, tag="os",
                                              name=f"os_{itt}")
                os_ = os_tiles[itt]
                nc.vector.tensor_copy(os_[:, dsl], pso[:])
                if dh == 1:
                    nc.sync.dma_start(out_d[itt * P:(itt + 1) * P, :], os_[:])

            def queue_outproj(it):
                for t in range(4):
                    for dh in range(2):
                        pending.append(
                            lambda pool=None, itt=it * 4 + t, d=dh:
                                outproj_group(itt, d, pool)
                        )

            def weave(units, jts):
                q = list(jts)
                for i, u in enumerate(units):
                    u()
                    if i < len(units) - 1 and q:
                        it_, p_, po_, jt_ = q.pop(0)
                        emit_jt(it_, p_, po_, jt_)
                for it_, p_, po_, jt_ in q:
                    emit_jt(it_, p_, po_, jt_)

            # ---- Prologue woven with the (0,0) j-loop. The first units use
            # the (still idle) sim pool so they double-buffer.
            kv_unit(0, ps_sim)
            for jt in range(4):
                v_unit(jt, ps_sim)
            qt_proj(0, 0, ps_sim)
            a0 = alloc_po(0, 0)
            weave(
                [lambda: kv_unit(1, ps_small)]
                + [lambda j=j: v_unit(j, ps_small) for j in range(4, 8)],
                [(0, 0, a0, jt) for jt in range(0, 4)],
            )
            weave(
                [lambda: kv_unit(2, ps_small)]
                + [lambda j=j: v_unit(j, ps_small) for j in range(8, 12)]
                + [lambda: qt_proj(1, 0, ps_small)],
                [(0, 0, a0, jt) for jt in range(4, 8)],
            )
            weave(
                [lambda: kv_unit(3, ps_small)]
                + [lambda j=j: v_unit(j, ps_small) for j in range(12, 16)],
                [(0, 0, a0, jt) for jt in range(8, 12)],
            )
            for jt in range(12, 16):
                emit_jt(0, 0, a0, jt)

            # ---- Remaining j-loops: one continuous pipeline. The previous
            # chunk's normalize is emitted after the next chunk's first jt so
            # its last oT (still in the skew queue) lands first. qT for the
            # next chunk is dripped as 2-kt chunks from jt 2.
            seq = [(it, p) for it in range(IT) for p in range(PAIRS)]
            qt_done = {(0, 0), (1, 0)}  # (p, it) pairs done/queued
            prev = (0, 0, a0)
            for i, (it, p) in enumerate(seq[1:], 1):
                po = alloc_po(it, p)
                for jt in range(NT):
                    emit_jt(it, p, po, jt)
                    if jt == 0 and prev is not None:
                        pit, pp, ppo = prev
                        emit_normalize(pit, pp, ppo)
                        if pp == PAIRS - 1:
                            queue_outproj(pit)
                        if (it, p) == (IT - 1, PAIRS - 1):
                            for t in range(4):
                                for dh in range(2):
                                    pending.append(
                                        lambda pool=None, itt=(IT - 1) * 4 + t,
                                        d=dh: outproj_c0(itt, d, pool)
                                    )
                    if jt == 1 and i + 1 < len(seq):
                        nit, np_ = seq[i + 1]
                        if (np_, nit) not in qt_done:
                            qt_proj_split(np_, nit)
                            qt_done.add((np_, nit))
                prev = (it, p, po)
            flush_skew()
            for idx, fn in enumerate(pending):
                fn(ps_sim if idx % 2 else None)
            del pending[:]
            poss = emit_normalize(IT - 1, PAIRS - 1, prev[2], defer=False)
            # Keep the PE's HAM clock gate warm through the normalize chain
            # (Vector-bound, ~3us) so the final projections run at 2.4 GHz. The
            # dummies read the just-copied accumulators so they execute inside
            # that window rather than draining early.
            for d in range(6):
                psd = ps_sim.tile([P, 512], f32, tag="sim", name=f"psd{d}")
                nc.tensor.matmul(psd[:], kT2[0:64, 0:P],
                                 poss[d % 2][0:64, :],
                                 start=True, stop=True)

            for t in range(4):
                for dh in range(2):
                    outproj_fin(
                        (IT - 1) * 4 + t, dh,
                        ps_sim if (t + dh) % 2 else None,
                    )
            for fn in pending:
                fn()
            del pending[:]

    nc.compile()
    return nc


_NC = None


def _get_nc():
    global _NC
    if _NC is None:
        _NC = _build()
    return _NC


def _prep_in_maps(x, Wq, Wkv, Wout):
    import ml_dtypes

    bf16 = ml_dtypes.bfloat16
    wk = Wkv[:, 0:DIM_HEAD]
    wv = Wkv[:, DIM_HEAD:]
    wkk = (
        np.concatenate([wk, wk], axis=1)
        .reshape(KT, P, P)
        .transpose(1, 0, 2)
        .astype(bf16)
    )
    wv_p = np.ascontiguousarray(
        wv.reshape(KT, P, DIM_HEAD).transpose(1, 0, 2).astype(bf16)
    )
    xt_cache = {}
    in_maps = []
    for core in range(N_CORES):
        b, g4 = divmod(core, 4)
        if b not in xt_cache:
            xt_cache[b] = np.ascontiguousarray(
                x[b].T.reshape(KT, P, IT, 512).transpose(1, 2, 0, 3).astype(bf16)
            )
        h0 = g4 * HEADS_PER_CORE
        wq_p = np.ascontiguousarray(
            np.transpose(Wq[h0:h0 + HEADS_PER_CORE], (1, 0, 2))
            .reshape(DIM, INNER)
            .reshape(KT, P, INNER)
            .transpose(1, 0, 2)
            .astype(bf16)
        )
        wout_p = np.ascontiguousarray(
            Wout[h0 * DIM_HEAD:(h0 + HEADS_PER_CORE) * DIM_HEAD]
            .reshape(CHUNKS, P, DIM)
            .transpose(1, 0, 2)
            .astype(bf16)
        )
        boot = np.ascontiguousarray(
            np.concatenate(
                [wkk.reshape(P, KT * P),
                 xt_cache[b][:, 0].reshape(P, KT * 512)],
                axis=1,
            )
        )
        in_maps.append(
            {
                "xt": xt_cache[b],
                "boot": boot,
                "wv": wv_p,
                "wq": wq_p,
                "wout": wout_p,
            }
        )
    return in_maps


def _ensure_hook_shim():
    """bass_utils imports antenv.axon_hooks when tracing is requested via
    env (BASS_TRACE); that module is absent on this image. Provide a no-op
    fallback so an inherited env var cannot break a plain run."""
    try:
        import antenv.axon_hooks  # noqa: F401
    except Exception:
        import sys
        import types

        m = types.ModuleType("antenv.axon_hooks")
        m.get_axon_ntff_profile_hook = lambda: None
        m.set_axon_ntff_profile_hook = lambda h: None
        sys.modules["antenv.axon_hooks"] = m


def run(inputs, trace=False):
    """Run on 8 cores; returns (full_output, BassKernelResults)."""
    _ensure_hook_shim()
    nc = _get_nc()
    in_maps = _prep_in_maps(
        np.asarray(inputs["x"]),
        np.asarray(inputs["Wq"]),
        np.asarray(inputs["Wkv"]),
        np.asarray(inputs["Wout"]),
    )
    res = run_bass_kernel_spmd(
        nc, in_maps, core_ids=list(range(N_CORES)), trace=trace
    )
    out = np.zeros((B, N, DIM), dtype=np.float32)
    for core in range(N_CORES):
        b = core // 4
        out[b] += res.results[core]["out"].astype(np.float32)
    return out, res


def kernel(**inputs) -> np.ndarray:
    out, _ = run(inputs, trace=bool(os.environ.get("BASS_KERNEL_TRACE")))
    return out
